# revision 40
# baseline (speedup 1.0000x reference)
"""Trainium2 Bass kernel for the MemoryEfficientMambaBlock problem.

Data-parallel over 8 NeuronCores: x sharded over tokens, small weights
replicated. Per core, per 448-token tile (14 tiles exactly cover the
6272 tokens/core; tokens grouped 4x112 so the DMA-XBAR transpose tiles
align):
  LayerNorm (bn_stats token-major, batched sqrt on ACT) -> DVE
  tensor_scalar writes xhat in bf16 (gamma folded into W_proj, beta into
  the proj bias) -> DMA-XBAR transpose to feature-major bf16 (no PE
  transposes) -> bf16 matmul x@W_projT with SiLU+bias fused in the ACT
  copyback -> bf16 matmul @W_stateT -> SiLU+(b_state+initial_state) ->
  K=9 bf16 matmul (ones row carries b_out), residual added in place into
  the x tile on DVE.

Pipelining: the LN+transpose chain for tile i+2 runs during tile i
(2-deep, so the PE never waits on it), x tiles DMA as halves on both
HWDGE queues, y writes go out on the gpsimd queue, and tile i-1's eight
matmul-3's are interleaved between tile i's matmul-1 m-steps so their
PSUM drains (DVE residual adds) never pace the PE.
"""

import sys

if "/opt/trn_rl_repo" not in sys.path:
    sys.path.insert(0, "/opt/trn_rl_repo")

import ml_dtypes
import numpy as np

import concourse.bass as bass
import concourse.mybir as mybir
import concourse.tile as tile
from concourse.bass_utils import run_bass_kernel_spmd

P = 128
PG = 112  # tokens per partition-group (multiple of 16 for the XBAR)
G = 4  # groups per tile
TILE_T = PG * G  # 448
D_MODEL = 1024
D_INNER = 2048
D_STATE = 8
EPS = 1e-5
N_CORES = 8
TOK_TOTAL = 2 * 128 * 196  # 50176
TOK = TOK_TOTAL // N_CORES  # 6272
NT = TOK // TILE_T  # 14 tiles exactly

KD = D_MODEL // P  # 8 contraction chunks for matmul 1
ME = D_INNER // P  # 16 output tiles for matmul 1 / contraction chunks for 2

F32 = mybir.dt.float32
BF16 = mybir.dt.bfloat16


def _split_multi_waits(nc):
    """This container's walrus accepts at most ONE semaphore wait per
    instruction. Hoist all but the last wait of each instruction onto
    fresh single-wait NoOps inserted immediately before it on the same
    engine (the sequencer processes instructions in order, so semantics
    are unchanged)."""
    n_split = 0
    for f in nc.m.functions:
        for blk in f.blocks:
            out = []
            changed = False
            for inst in blk.instructions:
                si = inst.sync_info
                waits = list(si.on_wait) if si is not None else []
                if len(waits) > 1:
                    changed = True
                    for j, w in enumerate(waits[:-1]):
                        nop = mybir.InstNoOp(
                            name=f"{inst.name}-wsplit{j}", ins=[], outs=[]
                        )
                        nop.engine = inst.engine
                        nop.sync_info = mybir.SyncInfo(on_wait=[w], on_update=[])
                        out.append(nop)
                        n_split += 1
                    inst.sync_info = mybir.SyncInfo(
                        on_wait=[waits[-1]], on_update=list(si.on_update)
                    )
                out.append(inst)
            if changed:
                blk.instructions = out
    return n_split


def build_kernel():
    nc = bass.Bass()
    x = nc.dram_tensor("x", [TOK, D_MODEL], F32, kind="ExternalInput")
    # [m, p, c, j] = (W_proj*gamma)[m*128+j, c*128+p]
    wpt = nc.dram_tensor("wpt", [ME, P, KD, P], BF16, kind="ExternalInput")
    wst = nc.dram_tensor("wst", [P, ME, D_STATE], BF16, kind="ExternalInput")
    wo9 = nc.dram_tensor("wo9", [D_STATE + 1, D_MODEL], BF16, kind="ExternalInput")
    bpm = nc.dram_tensor("bpm", [P, ME], F32, kind="ExternalInput")
    b2 = nc.dram_tensor("b2", [D_STATE, 1], F32, kind="ExternalInput")
    y = nc.dram_tensor("y", [TOK, D_MODEL], F32, kind="ExternalOutput")

    with tile.TileContext(nc) as tc:
        with (
            tc.tile_pool(name="singles", bufs=1) as singles,
            tc.tile_pool(name="xpool", bufs=3) as xpool,
            tc.tile_pool(name="outp", bufs=2) as outp,
            tc.tile_pool(name="xnpool", bufs=2) as xnpool,
            tc.tile_pool(name="xtpool", bufs=3) as xtpool,
            tc.tile_pool(name="projp", bufs=2) as projp,
            tc.tile_pool(name="statp", bufs=6) as statp,
            tc.tile_pool(name="ps1", bufs=4, space="PSUM") as ps1,
            tc.tile_pool(name="ps2", bufs=1, space="PSUM") as ps2,
            tc.tile_pool(name="ps3", bufs=2, space="PSUM") as ps3,
            tc.tile_pool(name="psw", bufs=1, space="PSUM") as psw,
        ):

            # ring assignment: sync carries ONLY the XBAR transposes (they
            # must not share a ring position behind bulk transfers, and
            # concurrent XBAR use from two rings corrupts data); scalar
            # carries x-in (+ weights at startup); gpsimd carries y-out.
            def a_dma(i, eng=None, quarters=False):
                off = i * TILE_T
                x_sb = xpool.tile([PG, G, D_MODEL], F32, tag="x")
                eng = eng or nc.scalar
                if quarters:
                    for g in range(G):
                        eng.dma_start(
                            x_sb[:, g], x[off + g * PG : off + (g + 1) * PG, :]
                        )
                else:
                    half = TILE_T // 2
                    eng.dma_start(
                        x_sb[:, 0:2],
                        x[off : off + half, :].rearrange("(g p) d -> p g d", p=PG),
                    )
                    eng.dma_start(
                        x_sb[:, 2:4],
                        x[off + half : off + TILE_T, :].rearrange(
                            "(g p) d -> p g d", p=PG
                        ),
                    )
                return x_sb

            # tile 0 as quarters on the otherwise-empty sync ring: stats
            # can start as each group lands. Tile 1 goes out on the scalar
            # ring BEFORE the weight slices: the weights aren't needed
            # until matmul-1 starts (~33us), while x1 gates tile 1's whole
            # LN chain, and the weight stream also congests the DMA fabric
            # exactly when the tile-0 transposes need it.
            x_tiles = [a_dma(0, eng=nc.sync, quarters=True), a_dma(1)]

            wst_sb = singles.tile([P, ME, D_STATE], BF16)
            nc.scalar.dma_start(wst_sb, wst[:, :])
            wo9_sb = singles.tile([D_STATE + 1, D_MODEL], BF16)
            nc.scalar.dma_start(wo9_sb, wo9[:, :])
            bpm_sb = singles.tile([P, ME], F32)
            nc.scalar.dma_start(bpm_sb, bpm[:, :])
            b2_sb = singles.tile([D_STATE, 1], F32)
            nc.scalar.dma_start(b2_sb, b2[:, :])
            wpt_sb = singles.tile([P, ME, KD, P], BF16)
            for m in range(ME):
                nc.scalar.dma_start(wpt_sb[:, m], wpt[m])
            eps_sb = singles.tile([PG, 1], F32)
            nc.vector.memset(eps_sb, EPS)
            # dummy sqrt: pulls the ACT Sqrt-table load off the LN
            # critical path (runs at t~7us while the ACT queue is idle)
            warm_sb = singles.tile([PG, 1], F32)
            nc.scalar.activation(
                warm_sb, eps_sb, mybir.ActivationFunctionType.Sqrt, bias=eps_sb
            )
            cs9_bufs = [
                singles.tile(
                    [D_STATE + 1, TILE_T], BF16, tag=f"cs9{j}", name=f"cs9{j}"
                )
                for j in range(2)
            ]
            for j in range(2):
                # whole-tile memset (partition-8-based APs are illegal);
                # rows 0..7 get overwritten by the SiLU each tile, row 8
                # stays 1.0 and multiplies the b_out row of wo9
                nc.gpsimd.memset(cs9_bufs[j], 1.0)

            # ~28us of throwaway matmuls on constant data into a spare
            # PSUM bank: keeps the PE's HAM clock-gate warm through the
            # startup window (x0 DMA + LN + transpose chain, ~48us) so the
            # first real tiles run at 2.4GHz instead of cold 1.2GHz. They
            # sit ahead of matmul-1 in PE program order and drain by ~40us.
            warm_ps = psw.tile([P, TILE_T], F32, name="warm_ps")
            for _ in range(150):
                nc.tensor.matmul(
                    warm_ps,
                    lhsT=cs9_bufs[0][:, 0:P],
                    rhs=cs9_bufs[1][:, :],
                    start=True,
                    stop=True,
                )

            def a_stats(x_sb):
                """bn stats for all 4 groups -> [PG, G, 2] mean/var"""
                mvt = statp.tile([PG, G, 2], F32, tag="mv")
                for g in range(G):
                    stats = statp.tile([PG, 2, 6], F32, tag="bnst")
                    nc.vector.bn_stats(stats[:, 0, :], x_sb[:, g, 0:512])
                    nc.vector.bn_stats(stats[:, 1, :], x_sb[:, g, 512:1024])
                    nc.vector.bn_aggr(mvt[:, g], stats)
                return mvt

            def a_norm(x_sb, mvt):
                """rstd (batched sqrt) + normalize -> bf16 token-major"""
                rstd = statp.tile([PG, G], F32, tag="rstd")
                nc.scalar.activation(
                    rstd,
                    mvt[:, :, 1],
                    mybir.ActivationFunctionType.Sqrt,
                    bias=eps_sb,
                )
                nc.vector.reciprocal(rstd, rstd)
                xn = xnpool.tile([PG, G, D_MODEL], BF16, tag="xn")
                for g in range(G):
                    nc.vector.tensor_scalar(
                        out=xn[:, g],
                        in0=x_sb[:, g],
                        scalar1=mvt[:, g, 0:1],
                        scalar2=rstd[:, g : g + 1],
                        op0=mybir.AluOpType.subtract,
                        op1=mybir.AluOpType.mult,
                    )
                return xn

            def a_tr(xn):
                """DMA-XBAR transpose to feature-major: [p, g, c, t] with
                feature d = c*128 + p. All transposes stay on the sync
                ring: concurrent XBAR transposes from two rings corrupt
                each other (the XBAR tile buffer is shared)."""
                xnT = xtpool.tile([P, G, KD, PG], BF16, tag="xnT")
                for g in range(G):
                    nc.sync.dma_start_transpose(xnT[:, g], xn[:, g, :])
                return xnT

            def m3_step(pcs9, px_sb, pout, g):
                """one group of the previous tile's matmul 3 + residual"""
                for h in range(2):
                    p3 = ps3.tile([PG, 512], F32, tag="p3")
                    nc.tensor.matmul(
                        p3,
                        lhsT=pcs9[:, g * PG : (g + 1) * PG],
                        rhs=wo9_sb[:, h * 512 : (h + 1) * 512],
                        start=True,
                        stop=True,
                    )
                    nc.vector.tensor_add(
                        out=pout[:, g, h * 512 : (h + 1) * 512],
                        in0=p3,
                        in1=px_sb[:, g, h * 512 : (h + 1) * 512],
                    )

            def y_dma(pout, poff):
                nc.gpsimd.dma_start(
                    y[poff : poff + TILE_T, :].rearrange("(g p) d -> p g d", p=PG),
                    pout,
                )

            # prologue: LN chain for tile 0 while weights stream on the
            # scalar ring; x1 lands behind the weights, its LN follows
            mvt0 = a_stats(x_tiles[0])
            xnT_q = [None, None]  # xnT for tiles i, i+1 relative to loop
            xn0 = a_norm(x_tiles[0], mvt0)
            xnT_q[0] = a_tr(xn0)
            mvt1 = a_stats(x_tiles[1])
            xn1 = a_norm(x_tiles[1], mvt1)
            xnT_q[1] = a_tr(xn1)

            prev = None  # (cs9, x_sb, off) of tile i-1

            for i in range(NT):
                off = i * TILE_T
                x_sb = x_tiles[i]
                xnT = xnT_q[0]
                xnT_q[0] = xnT_q[1]
                cs9 = cs9_bufs[i % 2]
                pout = (
                    outp.tile([PG, G, D_MODEL], F32, tag="out", name="pout")
                    if prev
                    else None
                )

                # matmul 1 m-steps with the previous tile's matmul-3
                # groups interleaved between them
                projT = projp.tile([P, ME, TILE_T], BF16, tag="projT")
                for m in range(ME):
                    p1 = ps1.tile([P, TILE_T], F32, tag="p1")
                    for c in range(KD):
                        nc.tensor.matmul(
                            p1,
                            lhsT=wpt_sb[:, m, c, :],
                            rhs=xnT[:, :, c, :],
                            start=(c == 0),
                            stop=(c == KD - 1),
                        )
                    nc.scalar.activation(
                        out=projT[:, m],
                        in_=p1,
                        func=mybir.ActivationFunctionType.Silu,
                        bias=bpm_sb[:, m : m + 1],
                        scale=1.0,
                    )
                    if prev is not None and m % 2 == 1 and m // 2 < G:
                        m3_step(prev[0], prev[1], pout, m // 2)
                if prev is not None:
                    y_dma(pout, prev[2])

                # x two tiles ahead: emitted after the residual reads of
                # x(i-1) so the pool's write-after-read is well ordered
                if i + 2 < NT:
                    x_tiles.append(a_dma(i + 2))

                # LN chain for tile i+2 (2-deep pipeline); the sqrt sits
                # on the ACT queue right after the m-step SiLUs, so its
                # bn_aggr dependency is long satisfied when reached
                if i + 2 < NT:
                    mvt_n = a_stats(x_tiles[i + 2])
                    xn_n = a_norm(x_tiles[i + 2], mvt_n)
                    xnT_q[1] = a_tr(xn_n)

                # matmul 2 + cs9 SiLU
                p2 = ps2.tile([D_STATE, TILE_T], F32, tag="p2")
                for k2 in range(ME):
                    nc.tensor.matmul(
                        p2,
                        lhsT=wst_sb[:, k2, :],
                        rhs=projT[:, k2, :],
                        start=(k2 == 0),
                        stop=(k2 == ME - 1),
                    )
                nc.scalar.activation(
                    out=cs9[:D_STATE, :],
                    in_=p2,
                    func=mybir.ActivationFunctionType.Silu,
                    bias=b2_sb,
                    scale=1.0,
                )
                prev = (cs9, x_sb, off)

            # epilogue: matmul 3 of the last tile; per-group y writes
            # spread over the three DMA rings so the tail is short
            pcs9, px_sb, poff = prev
            pout = outp.tile([PG, G, D_MODEL], F32, tag="out")
            tail_engs = [nc.sync, nc.scalar, nc.gpsimd, nc.sync]
            for g in range(G):
                m3_step(pcs9, px_sb, pout, g)
                tail_engs[g].dma_start(
                    y[poff + g * PG : poff + (g + 1) * PG, :], pout[:, g]
                )

    _split_multi_waits(nc)
    return nc


_NC_CACHE = None


def _get_nc():
    global _NC_CACHE
    if _NC_CACHE is None:
        _NC_CACHE = build_kernel()
    return _NC_CACHE


def make_in_maps(inputs):
    x = np.ascontiguousarray(inputs["x"], dtype=np.float32).reshape(-1, D_MODEL)
    W_proj = np.asarray(inputs["W_proj"], dtype=np.float32)
    b_proj = np.asarray(inputs["b_proj"], dtype=np.float32)
    W_state = np.asarray(inputs["W_state"], dtype=np.float32)
    b_state = np.asarray(inputs["b_state"], dtype=np.float32)
    W_out = np.asarray(inputs["W_out"], dtype=np.float32)
    b_out = np.asarray(inputs["b_out"], dtype=np.float32)
    initial_state = np.asarray(inputs["initial_state"], dtype=np.float32)
    gamma = np.asarray(inputs["gamma"], dtype=np.float32)
    beta = np.asarray(inputs["beta"], dtype=np.float32)

    # gamma folds into W_proj, beta into the proj bias
    Wg = W_proj * gamma[None, :]
    bvec = b_proj + W_proj @ beta
    # [m, p, c, j] = Wg[m*128+j, c*128+p]
    wpt_host = np.ascontiguousarray(
        Wg.reshape(ME, P, KD, P).transpose(0, 3, 2, 1)
    ).astype(ml_dtypes.bfloat16)
    wst_host = np.ascontiguousarray(
        W_state.T.reshape(ME, P, D_STATE).transpose(1, 0, 2)
    ).astype(ml_dtypes.bfloat16)

    shared = {
        "wpt": wpt_host,
        "wst": wst_host,
        "wo9": np.ascontiguousarray(
            np.concatenate([W_out.T, b_out[None, :]], axis=0)
        ).astype(ml_dtypes.bfloat16),
        "bpm": np.ascontiguousarray(bvec.reshape(ME, P).T),
        "b2": np.ascontiguousarray(
            (b_state + initial_state.reshape(-1)).reshape(D_STATE, 1)
        ),
    }
    in_maps = []
    for c in range(N_CORES):
        m = {"x": np.ascontiguousarray(x[c * TOK : (c + 1) * TOK])}
        m.update(shared)
        in_maps.append(m)
    return in_maps


def kernel(**inputs) -> np.ndarray:
    nc = _get_nc()
    in_maps = make_in_maps(inputs)
    res = run_bass_kernel_spmd(nc, in_maps, core_ids=list(range(N_CORES)))
    out = np.concatenate([res.results[c]["y"] for c in range(N_CORES)], axis=0)
    return out.reshape(np.asarray(inputs["x"]).shape)
